# revision 32
# baseline (speedup 1.0000x reference)
"""Trainium2 Bass kernel for nn_Attention (B=4, N=2048, C=1024, H=16, D=64).

Sharding: 8 cores; core c handles batch b=c//2 and heads [8*(c%2), 8*(c%2)+8).
Each core computes qkv projection for its 512 channels, RMSNorm(q/k),
attention over its 8 heads, and a partial output projection (contraction over
its 512 channels). Host sums the two partial proj outputs per batch.

All matmuls run as float32r (fp32 with 11-bit mantissa, full PE rate at
free-dim>=256). Host pre-rounds DRAM-fed matmul operands; on-device
intermediates are rounded by the producing DVE/ACT instruction writing to
float32r-typed tiles.

Layouts per core:
  xT      [1024, 2048]  x[b] transposed (C on partitions)
  qT/kT   [128, 2048] per head-pair (2 heads x 64ch on partitions, tokens free)
  v       [2048, 528]   natural, 4 pairs x [64v | 1 one | 1 pad] x 2 heads
  scores  sT[j, i] per head; softmax over j (partitions) via ones-column in v
  outT    [512, 2048]   attention output transposed -> proj lhsT
"""

import os
import numpy as np
import ml_dtypes

B, N, C, H, D = 4, 2048, 1024, 16, 64
NCORES = 8
HPC = 8           # heads per core
CH = HPC * D      # 512 channels per core
VSEG = 2 * D + 4  # 132 cols per pair in v_aug: [64 v | 1 | 1][64 v | 1 | 1]
VW = 4 * VSEG     # 528
EPS = 1e-6

_CACHE = {}
LAST_RESULT = [None]


def _round_f32r(x):
    x = np.ascontiguousarray(x, dtype=np.float32)
    u = x.view(np.uint32)
    keep = np.uint32(0xFFFFF000)
    half = np.uint32(0x800)
    lsb = (u >> np.uint32(12)) & np.uint32(1)
    r = (u + (half - np.uint32(1)) + lsb) & keep
    return r.view(np.float32)


def _build_nc():
    import concourse.tile as tile
    import concourse.mybir as mybir
    from concourse import bacc

    F32 = mybir.dt.float32
    F32R = mybir.dt.float32r
    BF16 = mybir.dt.bfloat16
    AF = mybir.ActivationFunctionType

    nc = bacc.Bacc("TRN2", target_bir_lowering=False, debug=False,
                   num_devices=NCORES)

    XTB = nc.dram_tensor("XTB", [C, N], BF16, kind="ExternalInput")
    WQ = nc.dram_tensor("WQ", [C, CH], BF16, kind="ExternalInput")
    WK = nc.dram_tensor("WK", [C, CH], BF16, kind="ExternalInput")
    WVA = nc.dram_tensor("WVA", [C, VW], BF16, kind="ExternalInput")
    WP = nc.dram_tensor("WP", [CH, C], F32R, kind="ExternalInput")
    BQK = nc.dram_tensor("BQK", [128, 8], F32, kind="ExternalInput")
    BVA = nc.dram_tensor("BVA", [128, VW], F32, kind="ExternalInput")
    BP = nc.dram_tensor("BP", [128, C], F32, kind="ExternalInput")
    QKN = nc.dram_tensor("QKN", [128, 2], F32, kind="ExternalInput")
    BLK = nc.dram_tensor("BLK", [128, 130], F32R, kind="ExternalInput")
    SEL = nc.dram_tensor("SEL", [65, 128], F32R, kind="ExternalInput")
    ONESB = nc.dram_tensor("ONESB", [65, 64], F32R, kind="ExternalInput")
    Y = nc.dram_tensor("Y", [N, C], F32, kind="ExternalOutput")

    NT = N // 128          # 16 token tiles
    KT = C // 128          # 8 contraction tiles
    NCHUNK = N // 512      # 4 chunks of 512 tokens

    with tile.TileContext(nc) as tc:
        from contextlib import ExitStack
        with ExitStack() as ctx:
            const_p = ctx.enter_context(tc.tile_pool(name="const", bufs=1))
            xtb_p = ctx.enter_context(tc.tile_pool(name="xtb", bufs=8))
            outT_p = ctx.enter_context(tc.tile_pool(name="outT", bufs=4))

            big_ps = ctx.enter_context(
                tc.tile_pool(name="big", bufs=3, space="PSUM"))
            po_ps = ctx.enter_context(
                tc.tile_pool(name="po", bufs=2, space="PSUM"))

            # constants
            bqk_sb = const_p.tile([128, 8], F32, tag="bqk")
            nc.sync.dma_start(bqk_sb[:], BQK.ap()[:, :])
            bva_sb = const_p.tile([128, VW], F32, tag="bva")
            nc.sync.dma_start(bva_sb[:], BVA.ap()[:, :])
            bp_sb = const_p.tile([128, C], F32, tag="bp")
            nc.sync.dma_start(bp_sb[:], BP.ap()[:, :])
            qkn_sb = const_p.tile([128, 2], F32, tag="qkn")
            nc.sync.dma_start(qkn_sb[:], QKN.ap()[:, :])
            blk_sb = const_p.tile([128, 130], F32R, tag="blk")
            nc.sync.dma_start(blk_sb[:], BLK.ap()[:, :])
            sel_sb = const_p.tile([65, 128], F32R, tag="sel")
            nc.sync.dma_start(sel_sb[:], SEL.ap()[:, :])
            onesb_sb = const_p.tile([65, 64], F32R, tag="onesb")
            nc.sync.dma_start(onesb_sb[:], ONESB.ap()[:, :])
            eps_q = const_p.tile([128, 1], F32, tag="eps_q")
            nc.vector.memset(eps_q[:], float(EPS))
            eps_k = const_p.tile([128, 1], F32, tag="eps_k")
            nc.vector.memset(eps_k[:], float(EPS * 64))

            xtb_tiles = []
            for kt in range(KT):
                t = xtb_p.tile([128, N], BF16, tag="xtb", name=f"xtb{kt}")
                nc.sync.dma_start(t[:], XTB.ap()[kt * 128:(kt + 1) * 128, :])
                xtb_tiles.append(t)

            # ---------------- Phase V: v for all pairs (bf16) -------------
            v_ctx = tc.tile_pool(name="v", bufs=16)
            v_p = v_ctx.__enter__()
            v_tiles = []
            for nt in range(NT):
                v_tiles.append(
                    v_p.tile([128, VW], BF16, tag="v", name=f"vt{nt}"))
            def emit_v_block(nt_range):
                for nt in nt_range:
                    for vh in range(2):
                        sl = slice(vh * (VW // 2), (vh + 1) * (VW // 2))
                        ps = big_ps.tile([128, VW // 2], F32, tag="big")
                        for kt in range(KT):
                            nc.tensor.matmul(
                                ps[:],
                                xtb_tiles[kt][:, nt * 128:(nt + 1) * 128],
                                wv_sb[:, kt * VW + vh * (VW // 2):
                                      kt * VW + (vh + 1) * (VW // 2)],
                                start=(kt == 0), stop=(kt == KT - 1))
                        nc.vector.tensor_add(
                            v_tiles[nt][:, sl], ps[:], bva_sb[:, sl])

            # ---------------- pipelined pair loop -------------------------
            pair_ctx = ExitStack()
            w_p = pair_ctx.enter_context(tc.tile_pool(name="w", bufs=1))
            qtb_p = pair_ctx.enter_context(tc.tile_pool(name="qtb", bufs=1))
            qpart_p = pair_ctx.enter_context(
                tc.tile_pool(name="qpart", bufs=1))
            sq_p = pair_ctx.enter_context(tc.tile_pool(name="sq", bufs=1))

            wv_ctx = tc.tile_pool(name="wv", bufs=1)
            wv_p = wv_ctx.__enter__()
            wv_sb = wv_p.tile([128, KT * VW], BF16, tag="wv")
            for kt in range(KT):
                nc.sync.dma_start(wv_sb[:, kt * VW:(kt + 1) * VW],
                                  WVA.ap()[kt * 128:(kt + 1) * 128, :])

            state = {}

            def emit_w_loads(hp):
                wq_sb = w_p.tile([128, C], BF16, tag="wq")
                wk_sb = w_p.tile([128, C], BF16, tag="wk")
                for kt in range(KT):
                    nc.sync.dma_start(
                        wq_sb[:, kt * 128:(kt + 1) * 128],
                        WQ.ap()[kt * 128:(kt + 1) * 128,
                                hp * 128:(hp + 1) * 128])
                    nc.sync.dma_start(
                        wk_sb[:, kt * 128:(kt + 1) * 128],
                        WK.ap()[kt * 128:(kt + 1) * 128,
                                hp * 128:(hp + 1) * 128])
                st = state[hp] = {}
                st["wq"], st["wk"] = wq_sb, wk_sb
                st["qT_b"] = qtb_p.tile([128, N], F32, tag="qtb",
                                        name=f"qTb{hp}")
                st["kT_b"] = qtb_p.tile([128, N], F32, tag="ktb",
                                        name=f"kTb{hp}")
                st["qpart"] = qpart_p.tile([128, N], F32, tag="qpart",
                                           name=f"qpart{hp}")
                st["kpart"] = qpart_p.tile([128, N], F32, tag="kpart",
                                           name=f"kpart{hp}")

            def emit_qkv_piece(hp, piece):
                # piece 0: q half0; 1: k half0; 2: q half1; 3: k half1
                st = state[hp]
                half, is_k = piece // 2, piece % 2
                wsb = st["wk"] if is_k else st["wq"]
                part = st["kpart"] if is_k else st["qpart"]
                dst = st["kT_b"] if is_k else st["qT_b"]
                bcol = (4 + hp) if is_k else hp
                for cp in range(2):
                    ps = big_ps.tile([128, 1024], F32, tag="big")
                    for sub in range(2):
                        c0 = cp * 1024 + sub * 512
                        for i in range(4):
                            kt = 4 * half + i
                            nc.tensor.matmul(
                                ps[:, sub * 512:(sub + 1) * 512],
                                wsb[:, kt * 128:(kt + 1) * 128],
                                xtb_tiles[kt][:, c0:c0 + 512],
                                start=(i == 0), stop=(i == 3))
                    csl = slice(cp * 1024, (cp + 1) * 1024)
                    if half == 0:
                        nc.vector.tensor_scalar(
                            part[:, csl], ps[:], bqk_sb[:, bcol:bcol + 1],
                            None, op0=mybir.AluOpType.add)
                    else:
                        nc.vector.tensor_add(dst[:, csl], ps[:], part[:, csl])

            def emit_stats_apply(hp):
                st = state[hp]
                qT_b, kT_b = st["qT_b"], st["kT_b"]
                qTn = qtn_p.tile([128, N], BF16, tag="qtn", name=f"qTn{hp}")
                kTn = qtn_p.tile([128, N], BF16, tag="ktn", name=f"kTn{hp}")
                st["qTn"], st["kTn"] = qTn, kTn
                lg_tiles = {}
                for (src_t, is_k) in ((qT_b, False), (kT_b, True)):
                    blk_cols = (blk_sb[:, 65:130] if is_k
                                else blk_sb[:, 0:65])
                    eps_ap = eps_k if is_k else eps_q
                    for cp in range(2):
                        vps = big_ps.tile([65, 1024], F32, tag="big")
                        for sub in range(2):
                            csl = slice(cp * 1024 + sub * 512,
                                        cp * 1024 + sub * 512 + 512)
                            sq = sq_p.tile([128, 512], F32R, tag="sq")
                            nc.vector.tensor_mul(sq[:], src_t[:, csl],
                                                 src_t[:, csl])
                            nc.tensor.matmul(
                                vps[:, sub * 512:(sub + 1) * 512],
                                blk_cols, sq[:], start=True, stop=True)
                        lg = stat_p.tile([65, 1024], F32, tag="stat",
                                         name=f"lg{hp}{cp}{int(is_k)}")
                        nc.scalar.activation(lg[:], vps[:], AF.Ln,
                                             bias=eps_ap[0:65, :])
                        lg_tiles[(is_k, cp)] = lg
                rs_tiles = {}
                for (is_k, cp), lg in lg_tiles.items():
                    rs = stat_p.tile([65, 1024], F32R, tag="stat",
                                     name=f"rs{hp}{cp}{int(is_k)}")
                    nc.scalar.activation(rs[:], lg[:], AF.Exp, scale=-0.5)
                    rs_tiles[(is_k, cp)] = rs
                for (src_t, dstn, wcol, is_k) in (
                        (qT_b, qTn, 0, False), (kT_b, kTn, 1, True)):
                    for chk in range(NCHUNK):
                        cp, sub = chk // 2, chk % 2
                        rs = rs_tiles[(is_k, cp)]
                        csl = slice(chk * 512, (chk + 1) * 512)
                        bc_ps = big_ps.tile([128, 512], F32, tag="big",
                                            name="bc_ps")
                        nc.tensor.matmul(
                            bc_ps[:], sel_sb[:],
                            rs[:, sub * 512:(sub + 1) * 512],
                            start=True, stop=True)
                        nc.vector.scalar_tensor_tensor(
                            dstn[:, csl], src_t[:, csl],
                            qkn_sb[:, wcol:wcol + 1], bc_ps[:],
                            op0=mybir.AluOpType.mult,
                            op1=mybir.AluOpType.mult)

            outT_tiles = []

            def emit_attention_start(hp):
                st = state[hp]
                outT = outT_p.tile([128, N], F32R, tag="outT",
                                   name=f"outT{hp}")
                outT_tiles.append(outT)
                st["outT"] = outT
                st["den"] = rcp_p.tile([8, 512], F32, tag="den_pack",
                                       bufs=2, name=f"den{hp}")
                st["po_sbs"] = []

            def emit_attention_ic(hp, ic):
                st = state[hp]
                qTn, kTn, outT = st["qTn"], st["kTn"], st["outT"]
                vbase = hp * VSEG
                isl = slice(ic * 512, (ic + 1) * 512)
                poA = po_ps.tile([65, 512], F32, tag="po")
                poB = po_ps.tile([65, 512], F32, tag="po")
                for jt in range(NT):
                    jsl = slice(jt * 128, (jt + 1) * 128)
                    sc = big_ps.tile([128, 1024], F32, tag="big")
                    nc.tensor.matmul(
                        sc[:, 0:512], kTn[0:64, jsl], qTn[0:64, isl],
                        start=True, stop=True, tile_position=(0, 0))
                    nc.tensor.matmul(
                        sc[:, 512:1024], kTn[64:128, jsl], qTn[64:128, isl],
                        start=True, stop=True, tile_position=(64, 0))
                    ex = ex_p.tile([128, 1024], BF16, tag="ex")
                    nc.scalar.activation(ex[:], sc[:], AF.Exp)
                    nc.tensor.matmul(
                        poA[:], v_tiles[jt][:, vbase:vbase + 65],
                        ex[:, 0:512], start=(jt == 0), stop=(jt == NT - 1))
                    nc.tensor.matmul(
                        poB[:],
                        v_tiles[jt][:, vbase + VSEG // 2:
                                    vbase + VSEG // 2 + 65],
                        ex[:, 512:1024], start=(jt == 0),
                        stop=(jt == NT - 1))
                for hh, (po, rowoff) in enumerate(((poA, 0), (poB, 64))):
                    idx = ic * 2 + hh
                    po_sb = rcp_p.tile([65, 512], F32, tag="po_sb",
                                       name=f"po_sb{hp}_{idx}", bufs=10)
                    nc.vector.tensor_copy(po_sb[:], po[:, :])
                    nc.sync.dma_start(st["den"][idx:idx + 1, :],
                                      po_sb[64:65, :])
                    st["po_sbs"].append((po_sb, rowoff, ic, idx))

            def emit_attention_end(hp):
                st = state[hp]
                outT = st["outT"]
                rcp_pack = rcp_p.tile([8, 512], F32R, tag="rcp_pack",
                                      bufs=2, name=f"rcpp{hp}")
                with nc.allow_low_precision(
                        reason="softmax denom recip rounded to f32r"):
                    nc.vector.reciprocal(rcp_pack[:], st["den"][:])
                rcp_al = rcp_p.tile([65, 1536], F32R, tag="rcp_al",
                                    bufs=1, name=f"rcpa{hp}")
                for idx in range(8):
                    r, fb = 32 * (idx % 3), 512 * (idx // 3)
                    nc.sync.dma_start(rcp_al[r:r + 1, fb:fb + 512],
                                      rcp_pack[idx:idx + 1, :])
                for (po_sb, rowoff, ic, idx) in st["po_sbs"]:
                    isl = slice(ic * 512, (ic + 1) * 512)
                    r, fb = 32 * (idx % 3), 512 * (idx // 3)
                    rb_ps = big_ps.tile([64, 512], F32, tag="big",
                                        name="rb_ps")
                    nc.tensor.matmul(rb_ps[:], onesb_sb[r:r + 1, :],
                                     rcp_al[r:r + 1, fb:fb + 512],
                                     start=True, stop=True)
                    nc.vector.tensor_mul(
                        outT[rowoff:rowoff + 64, isl], po_sb[0:64, :],
                        rb_ps[:])

            # software pipeline: qkv(p+1) pieces + stats inside
            # attention(p); attention_end(p) deferred into pair p+1
            emit_w_loads(0)
            emit_v_block(range(0, 6))
            emit_qkv_piece(0, 0)
            emit_qkv_piece(0, 1)
            emit_v_block(range(6, 12))
            emit_qkv_piece(0, 2)
            emit_qkv_piece(0, 3)
            emit_v_block(range(12, NT))
            wv_ctx.__exit__(None, None, None)
            qtn_p = pair_ctx.enter_context(tc.tile_pool(name="qtn", bufs=2))
            stat_p = pair_ctx.enter_context(tc.tile_pool(name="stat", bufs=5))
            rcp_p = pair_ctx.enter_context(tc.tile_pool(name="rcp", bufs=1))
            ex_p = pair_ctx.enter_context(tc.tile_pool(name="ex", bufs=2))
            emit_stats_apply(0)
            for hp in range(4):
                emit_attention_start(hp)
                if hp + 1 < 4:
                    emit_w_loads(hp + 1)
                for ic in range(NCHUNK):
                    emit_attention_ic(hp, ic)
                    if ic == 0 and hp > 0:
                        emit_attention_end(hp - 1)
                    if hp + 1 < 4:
                        if ic == 0:
                            emit_qkv_piece(hp + 1, 0)
                            emit_qkv_piece(hp + 1, 1)
                        elif ic == 1:
                            emit_qkv_piece(hp + 1, 2)
                            emit_qkv_piece(hp + 1, 3)
                        elif ic == 2:
                            emit_stats_apply(hp + 1)
            emit_attention_end(3)

            pair_ctx.close()
            v_ctx.__exit__(None, None, None)

            # ---------------- proj ---------------------------------------
            wp_p = ctx.enter_context(tc.tile_pool(name="wp", bufs=4))
            y_p = ctx.enter_context(tc.tile_pool(name="y", bufs=2))
            wp_tiles = []
            for kt in range(4):
                t = wp_p.tile([128, C], F32R, tag="wp")
                nc.sync.dma_start(t[:], WP.ap()[kt * 128:(kt + 1) * 128, :])
                wp_tiles.append(t)
            for nt in range(NT):
                ps = big_ps.tile([128, 1024], F32, tag="big")
                for sub in range(2):
                    for kt in range(4):
                        nc.tensor.matmul(
                            ps[:, sub * 512:(sub + 1) * 512],
                            outT_tiles[kt][:, nt * 128:(nt + 1) * 128],
                            wp_tiles[kt][:, sub * 512:(sub + 1) * 512],
                            start=(kt == 0), stop=(kt == 3))
                y_sb = y_p.tile([128, C], F32, tag="y")
                nc.vector.tensor_add(y_sb[:], ps[:], bp_sb[:])
                nc.sync.dma_start(Y.ap()[nt * 128:(nt + 1) * 128, :], y_sb[:])

    nc.compile()
    return nc


def _core_inputs(c, x, W_qkv, b_qkv, W_proj, b_proj, qn_w, kn_w):
    b, half = c // 2, c % 2
    hbase = HPC * half
    co = hbase * D                      # channel offset of this core's heads

    xT = np.ascontiguousarray(x[b].T, dtype=np.float32)
    WQc = W_qkv[:, co:co + CH].astype(ml_dtypes.bfloat16)
    WKc = W_qkv[:, C + co:C + co + CH].astype(ml_dtypes.bfloat16)
    WVc = W_qkv[:, 2 * C + co:2 * C + co + CH]
    WVA = np.zeros((C, VW), dtype=np.float32)
    BVA1 = np.zeros((VW,), dtype=np.float32)
    bv = b_qkv[2 * C + co:2 * C + co + CH]
    for hp in range(4):
        for hh in range(2):
            s = hp * VSEG + hh * (VSEG // 2)
            WVA[:, s:s + D] = WVc[:, (2 * hp + hh) * D:(2 * hp + hh + 1) * D]
            BVA1[s:s + D] = bv[(2 * hp + hh) * D:(2 * hp + hh + 1) * D]
            BVA1[s + D] = 1.0  # ones column for softmax denominators
    WVA = WVA.astype(ml_dtypes.bfloat16)
    BVA = np.broadcast_to(BVA1, (128, VW)).copy()

    BQK = np.zeros((128, 8), dtype=np.float32)
    for hp in range(4):
        BQK[:, hp] = b_qkv[co + hp * 128:co + (hp + 1) * 128]
        BQK[:, 4 + hp] = b_qkv[C + co + hp * 128:C + co + (hp + 1) * 128]

    WPc = _round_f32r(W_proj[co:co + CH, :])
    BP = (np.broadcast_to(b_proj, (128, C)).copy() if half == 0
          else np.zeros((128, C), dtype=np.float32))
    QKN = np.stack([np.tile(qn_w, 2), np.tile(kn_w, 2)], axis=1).astype(np.float32)
    BLK = np.zeros((128, 130), dtype=np.float32)
    BLK[0:64, 0] = 1.0 / D        # q head0 -> var row 0
    BLK[64:128, 64] = 1.0 / D     # q head1 -> var row 64
    BLK[0:64, 65] = 1.0           # k head0 (sumsq -> folds 1/8 into rsqrt)
    BLK[64:128, 129] = 1.0        # k head1
    BLK = _round_f32r(BLK)
    SEL = np.zeros((65, 128), dtype=np.float32)
    SEL[0, 0:64] = 1.0
    SEL[64, 64:128] = 1.0
    SEL = _round_f32r(SEL)
    ONESB = np.zeros((65, 64), dtype=np.float32)
    for r in (0, 32, 64):
        ONESB[r, :] = 1.0
    ONESB = _round_f32r(ONESB)

    xTb = xT.astype(ml_dtypes.bfloat16)
    return {"XTB": xTb, "WQ": WQc, "WK": WKc, "WVA": WVA, "WP": WPc,
            "BQK": BQK, "BVA": BVA, "BP": BP.astype(np.float32),
            "QKN": QKN, "BLK": BLK, "SEL": SEL, "ONESB": ONESB}


def kernel(x, W_qkv, b_qkv, W_proj, b_proj, qn_w, kn_w):
    from concourse.bass_utils import run_bass_kernel_spmd

    if "nc" not in _CACHE:
        _CACHE["nc"] = _build_nc()
    nc = _CACHE["nc"]

    args = (np.asarray(x, np.float32), np.asarray(W_qkv, np.float32),
            np.asarray(b_qkv, np.float32), np.asarray(W_proj, np.float32),
            np.asarray(b_proj, np.float32), np.asarray(qn_w, np.float32),
            np.asarray(kn_w, np.float32))
    in_maps = [_core_inputs(c, *args) for c in range(NCORES)]

    trace = os.environ.get("BASS_KERNEL_TRACE", "0") == "1"
    res = run_bass_kernel_spmd(nc, in_maps, core_ids=list(range(NCORES)),
                               trace=trace)
    LAST_RESULT[0] = res

    y = np.stack([res.results[2 * b]["Y"] + res.results[2 * b + 1]["Y"]
                  for b in range(B)])
    return y.astype(np.float32)
